# revision 9
# baseline (speedup 1.0000x reference)
"""Channel-attention kernel for Trainium2, SPMD across 8 NeuronCores.

Problem: x:[4,512,64,64] f32; q = wq@x+bq, k = wk@x+bk (Cq=64), v = wv@x+bv;
scores = q^T k -> [B,4096,4096]; attn = softmax(scores, -1);
out = v @ attn^T; y = gamma*out + x.

Sharding: 8 shards = 4 batches x 2 query-halves. Each core gets its batch's
x pre-rotated along the pixel axis so its 2048 queries sit in columns 0:2048
(softmax/AV are permutation-invariant over keys, so rotating keys/values is
harmless). This keeps the SPMD program identical on every core.

Per-core pipeline (fp8 DoubleRow on the PE wherever K>=256):
  1. x is loaded ONCE in fp8e4, split across DMA rings to reach the HBM
     read roofline: rows 256:512 through gpsimd casting DMAs, rows 0:256
     staged f32 on the sync ring and cast to fp8 by the scalar engine.
     Bulk transfers are split into 256-512KB DMAs so each ring keeps many
     in flight (single large DMAs run far below ring rate). wv and the
     small consts ride the scalar engine's DGE ring.
  2. QK projection in fp8 DR straight from xp (wq/wk/bq/bk pre-scaled x16
     on the host so fp8e4 weights avoid the subnormal range; the 256x
     score scale is removed for free by the exp activation's scale).
     The bias add writes q/k directly into their duplicated score operands
     (q2/k2), split per pixel half so half-0 score pairs start while the
     second half of x is still loading.
  3. V projection computed transposed (vT[m,c] = x^T wvT) in fp8 DR.
     The v bias is NOT added here: it contributes exactly gamma*bv to y
     (softmax rows sum to 1), folded into the output residual add via a
     host-precomputed gbv tile.
  4. ScoresT[m,n] = k^T q as K=64 pairs on disjoint row-groups (concurrent);
     exp on the scalar engine -> fp8e5, with scale=1/256 and bias=-4.
  5. Softmax denominators d[n] via an all-ones [128,2,128] DR stationary:
     each dps matmul broadcasts d[n] to all 128 partitions, directly
     usable in the [c,n] layout (reciprocal_approx_fast + gamma on DVE).
  6. AV directly in the residual layout: out[c,n] = sum_m vT[m,c] e[m,n]
     (lhsT = vP slice, rhs = expP). No transposes anywhere.
  7. y = av*(gamma/d) + gbv + x as tensor_mul + one fused
     scalar_tensor_tensor; x residual tiles are prefetched early and the
     output DMA is one wide transfer per group (per-tile for the last
     group to shorten the drain).

Precision: fp8 Q/K/V projections are well within the 2e-2 gate (errors
average out across the 4096-key softmax support and 512-channel
contractions); the residual path keeps x in exact fp32 end to end.
"""

import numpy as np

import concourse.bass as bass
import concourse.bacc as bacc
import concourse.mybir as mybir
import concourse.tile as tile
from concourse import bass_utils, masks

B, C, W, H = 4, 512, 64, 64
N = W * H          # 4096 pixels
CQ = 64            # query/key channels
NH = N // 2        # 2048 queries per core
NCORES = 8
F32 = mybir.dt.float32
BF16 = mybir.dt.bfloat16
FP8E4 = mybir.dt.float8e4
FP8E5 = mybir.dt.float8e5
DR = mybir.MatmulPerfMode.DoubleRow
ALU = mybir.AluOpType
VPAD = 528   # fp8 vT pair stride, %16 == 0
AF = mybir.ActivationFunctionType
WSCALE = 16.0          # host pre-scale on wq/wk/wv (and q/k biases)
ESCALE = 1.0 / (WSCALE * WSCALE)   # undone inside the exp activation

N_MT = N // 128    # 32 key tiles
N_G = NH // 512    # 4 query groups per core
NJ = N_MT // 2     # 16 fp8 pair tiles


def _emit(tc, x, wq, wk, wv, bqk, gbv, gamma, y):
    nc = tc.nc

    with (
        tc.tile_pool(name="const", bufs=1) as const,
        tc.tile_pool(name="data", bufs=1) as data,
        tc.tile_pool(name="stg", bufs=2) as stg,
        tc.tile_pool(name="outp", bufs=2) as outp,
        tc.tile_pool(name="ps_sc", bufs=2, space="PSUM") as ps_sc,
        tc.tile_pool(name="ps_av", bufs=2, space="PSUM") as ps_av,
        tc.tile_pool(name="ps_d", bufs=2, space="PSUM") as ps_d,
    ):
        # ---- scalar ring: weights + small consts (parallel with x) ----
        wqk_f = stg.tile([128, C], F32, tag="wqkf")
        nc.scalar.dma_start(wqk_f[0:CQ, :], wq)
        nc.scalar.dma_start(wqk_f[CQ:128, :], wk)
        bqk_s = const.tile([128, 1], F32, tag="bqk")
        nc.scalar.dma_start(bqk_s[:], bqk)
        wv_f = [stg.tile([128, C], F32, tag="wvf", bufs=4, name=f"wvf{r}")
                for r in range(4)]
        for r in range(4):
            nc.scalar.dma_start(wv_f[r][:], wv[r * 128:(r + 1) * 128, :])
        g_s = const.tile([1, 1], F32, tag="gs")
        nc.scalar.dma_start(g_s[:], gamma)
        gbv_s = const.tile([128, 4], F32, tag="gbv")
        nc.scalar.dma_start(gbv_s[:], gbv)

        # ---- x load: sync stages rows 0:256 f32 (16 small DMAs),
        #      gpsimd cast-DMAs rows 256:512 (8 DMAs). -------------------
        xp = [data.tile([128, 2 * N], FP8E4, tag=f"xp{pc}", name=f"xp{pc}")
              for pc in range(2)]
        xst = [data.tile([128, NH], F32, tag=f"xs{i}", name=f"xs{i}")
               for i in range(4)]
        for hh in range(2):
            for r in range(2):
                xs = xst[hh * 2 + r]
                for q in range(4):
                    nc.sync.dma_start(
                        xs[:, q * 512:(q + 1) * 512],
                        x[r * 128:(r + 1) * 128,
                          hh * NH + q * 512:hh * NH + (q + 1) * 512])
                for q in range(2):
                    nc.gpsimd.dma_start(
                        xp[1][:, (r % 2) * N + hh * NH + q * 1024:
                               (r % 2) * N + hh * NH + (q + 1) * 1024],
                        x[(r + 2) * 128:(r + 3) * 128,
                          hh * NH + q * 1024:hh * NH + (q + 1) * 1024])

        # ---- x residual prefetch (sync ring, after staging) ------------
        xres = [data.tile([128, 2048], F32, tag=f"xr{g}", name=f"xr{g}")
                for g in range(N_G)]
        for g in range(N_G):
            for q in range(2):
                nc.sync.dma_start(
                    xres[g][:, q * 1024:(q + 1) * 1024]
                    .rearrange("p (ct n) -> p ct n", ct=2),
                    x[q * 256:(q + 1) * 256, g * 512:(g + 1) * 512]
                    .rearrange("(ct p) n -> p ct n", ct=2))

        # ---- scalar engine casts the staged rows 0:256 to fp8 ----------
        for hh in range(2):
            for r in range(2):
                xs = xst[hh * 2 + r]
                for q in range(2):
                    nc.scalar.activation(
                        xp[0][:, (r % 2) * N + hh * NH + q * 1024:
                              (r % 2) * N + hh * NH + (q + 1) * 1024],
                        xs[:, q * 1024:(q + 1) * 1024], AF.Copy)

        # ---- constants -------------------------------------------------
        id_bf = const.tile([128, 128], BF16, tag="idb")
        masks.make_identity(nc, id_bf[:])
        ones_f32 = const.tile([1, 128], F32, tag="ones")
        nc.gpsimd.memset(ones_f32[:], 1.0)
        nbias = const.tile([128, 1], F32, tag="nbias")
        nc.gpsimd.memset(nbias[:], -4.0)
        onesDR = const.tile([128, 256], FP8E4, tag="onesDR")
        nc.gpsimd.memset(onesDR[:], 1.0)
        gammab = const.tile([128, 1], F32, tag="gammab")

        # ---- weight prep on PE (ps_sc slots, done before scores) -------
        pg = ps_sc.tile([128, 1], F32, tag="sc", name="pg")
        nc.tensor.matmul(pg[:], ones_f32[:], g_s[:], start=True, stop=True)
        nc.vector.tensor_copy(gammab[:], pg[:])

        # wq;wk stacked -> bf16 -> transpose -> fp8 DR pairs
        wqkb = stg.tile([128, C], BF16, tag="wqkb")
        nc.vector.tensor_copy(wqkb[:], wqk_f[:])
        wqkT_dr = [const.tile([128, 256], FP8E4, tag=f"wqkT{pc}",
                              name=f"wqkT{pc}") for pc in range(2)]
        for cc in range(4):
            pt = ps_sc.tile([128, 128], BF16, tag="sc", name=f"ptq{cc}")
            nc.tensor.transpose(pt[:], wqkb[:, cc * 128:(cc + 1) * 128],
                                id_bf[:])
            nc.vector.tensor_copy(
                wqkT_dr[cc // 2][:, (cc % 2) * 128:(cc % 2) * 128 + 128],
                pt[:])

        # wv -> bf16 -> wvT fp8 pairs
        wvTp = [const.tile([128, 1024], FP8E4, tag=f"wvTp{pc}",
                           name=f"wvTp{pc}") for pc in range(2)]
        wvb = []
        for r in range(4):
            wb = stg.tile([128, C], BF16, tag="wvb", bufs=4, name=f"wvb{r}")
            nc.vector.tensor_copy(wb[:], wv_f[r][:])
            wvb.append(wb)
        for cc in range(4):
            pt = ps_sc.tile([128, C], BF16, tag="sc", name=f"ptv{cc}")
            for r in range(4):
                nc.tensor.transpose(
                    pt[:, r * 128:(r + 1) * 128],
                    wvb[r][:, cc * 128:(cc + 1) * 128],
                    id_bf[:],
                )
            nc.vector.tensor_copy(
                wvTp[cc // 2][:, (cc % 2) * 512:(cc % 2) * 512 + 512],
                pt[:])

        # ---- QK + V projection, chasing the DMAs; q/k written directly
        #      into their duplicated score operands -----------------------
        q2 = data.tile([128, NH], BF16, tag="q2")
        k2 = data.tile([128, N], BF16, tag="k2")
        vP = [data.tile([128, 2 * VPAD], FP8E4, tag=f"vP{j}", name=f"vP{j}")
              for j in range(NJ)]

        def score_pair(expP_list, g, j):
            mA, mB = 2 * j, 2 * j + 1
            q_lo = q2[0:CQ, g * 512:(g + 1) * 512]
            q_hi = q2[CQ:128, g * 512:(g + 1) * 512]
            ps = ps_sc.tile([128, 1024], F32, tag="sc", name=f"ps{g}_{j}")
            nc.tensor.matmul(
                ps[:, 0:512], k2[0:CQ, mA * 128:(mA + 1) * 128], q_lo,
                start=True, stop=True,
            )
            nc.tensor.matmul(
                ps[:, 512:1024],
                k2[CQ:128, mB * 128:(mB + 1) * 128], q_hi,
                start=True, stop=True,
            )
            nc.scalar.activation(expP_list[j][:], ps[:], AF.Exp,
                                 bias=nbias[:], scale=ESCALE)

        expP = [data.tile([128, 1024], FP8E5, tag=f"expP{j}",
                          name=f"expP{j}_0", bufs=2) for j in range(NJ)]

        for hh in range(2):
            for gg in range(4):
                g = hh * 4 + gg
                ps = ps_av.tile([128, 512], F32, tag="av", name=f"qk{g}")
                for pc in range(2):
                    nc.tensor.matmul(
                        ps[:],
                        wqkT_dr[pc][:].rearrange("p (i n) -> p i n", i=2),
                        xp[pc][:].rearrange("p (i n) -> p i n", i=2)[
                            :, :, g * 512:(g + 1) * 512],
                        start=(pc == 0), stop=(pc == 1), perf_mode=DR,
                    )
                if hh == 0:
                    nc.vector.tensor_scalar_add(
                        q2[0:CQ, g * 512:(g + 1) * 512], ps[0:CQ, :],
                        bqk_s[0:CQ, :])
                nc.vector.tensor_scalar_add(
                    k2[0:CQ, g * 512:(g + 1) * 512], ps[CQ:128, :],
                    bqk_s[CQ:128, :])
            # duplicate into the second partition half
            nc.gpsimd.dma_start(
                k2[CQ:128, hh * NH:(hh + 1) * NH], k2[0:CQ, hh * NH:(hh + 1) * NH])
            if hh == 0:
                nc.gpsimd.dma_start(q2[CQ:128, :], q2[0:CQ, :])
                # half-0 score pairs can start while half 1 still loads
                for j in range(NJ // 2):
                    score_pair(expP, 0, j)
            # V projection for this half's key tiles
            for mt in range(hh * 16, hh * 16 + 16):
                ps = ps_d.tile([128, C], F32, tag="d", name=f"vps{mt}")
                for pc in range(2):
                    nc.tensor.matmul(
                        ps[:],
                        xp[pc][:].rearrange("p (i n) -> p i n", i=2)[
                            :, :, mt * 128:(mt + 1) * 128],
                        wvTp[pc][:].rearrange("p (i n) -> p i n", i=2),
                        start=(pc == 0), stop=(pc == 1), perf_mode=DR,
                    )
                j, half = divmod(mt, 2)
                nc.vector.tensor_copy(vP[j][:, half * VPAD:half * VPAD + C],
                                      ps[:])
        for j in range(NJ // 2, NJ):
            score_pair(expP, 0, j)

        # ---- attention -------------------------------------------------
        def alloc_expP(g):
            return [data.tile([128, 1024], FP8E5, tag=f"expP{j}",
                              name=f"expP{j}_{g}", bufs=2)
                    for j in range(NJ)]

        yv = y.rearrange("(ct p) n -> p ct n", ct=4)
        for g in range(N_G):
            nxt = alloc_expP(g + 1) if g + 1 < N_G else None

            # denominators broadcast to all partitions via all-ones DR
            dsum = ps_d.tile([128, 512], F32, tag="d", name=f"dsum{g}")
            ones_ap = onesDR[:].rearrange("p (i n) -> p i n", i=2)
            for j in range(NJ):
                nc.tensor.matmul(
                    dsum[:], ones_ap,
                    expP[j][:].rearrange("p (i n) -> p i n", i=2),
                    start=(j == 0), stop=(j == NJ - 1), perf_mode=DR,
                )
            dinv = data.tile([128, 512], F32, tag="dinv", bufs=2,
                             name=f"dinv{g}")
            nc.vector.reciprocal_approx_fast(dinv[:], dsum[:])
            nc.vector.tensor_scalar_mul(dinv[:], dinv[:], gammab[:])

            yow = outp.tile([128, 2048], F32, tag="yo", name=f"yo{g}")
            for ct in range(4):
                # interleave next group's scores to keep ACT fed
                if nxt is not None:
                    for j in range(ct * 4, ct * 4 + 4):
                        score_pair(nxt, g + 1, j)
                av = ps_av.tile([128, 512], F32, tag="av",
                                name=f"av{g}_{ct}")
                for j in range(NJ):
                    nc.tensor.matmul(
                        av[:],
                        vP[j][:].rearrange("p (i n) -> p i n", i=2)[
                            :, :, ct * 128:(ct + 1) * 128],
                        expP[j][:].rearrange("p (i n) -> p i n", i=2),
                        start=(j == 0), stop=(j == NJ - 1), perf_mode=DR,
                    )
                t0 = outp.tile([128, 512], F32, tag="ysc")
                nc.vector.tensor_mul(t0[:], av[:], dinv[:])
                yslice = yow[:, ct * 512:(ct + 1) * 512]
                nc.vector.scalar_tensor_tensor(
                    yslice, t0[:], gbv_s[:, ct:ct + 1],
                    xres[g][:, ct * 512:(ct + 1) * 512],
                    ALU.add, ALU.add)
                if g == N_G - 1:
                    # last group: drain per tile
                    nc.sync.dma_start(
                        y[ct * 128:(ct + 1) * 128, g * 512:(g + 1) * 512],
                        yslice)
            if g < N_G - 1:
                nc.sync.dma_start(
                    yv[:, :, g * 512:(g + 1) * 512],
                    yow[:].rearrange("p (ct n) -> p ct n", ct=4))
            expP = nxt


def build_nc():
    nc = bacc.Bacc("TRN2", target_bir_lowering=False, debug=False,
                   num_devices=NCORES)
    x = nc.dram_tensor("x", [C, N], F32, kind="ExternalInput")
    wq = nc.dram_tensor("wq", [CQ, C], F32, kind="ExternalInput")
    wk = nc.dram_tensor("wk", [CQ, C], F32, kind="ExternalInput")
    wv = nc.dram_tensor("wv", [C, C], F32, kind="ExternalInput")
    bqk = nc.dram_tensor("bqk", [128, 1], F32, kind="ExternalInput")
    gbv = nc.dram_tensor("gbv", [128, 4], F32, kind="ExternalInput")
    gamma = nc.dram_tensor("gamma", [1, 1], F32, kind="ExternalInput")
    y = nc.dram_tensor("y", [C, NH], F32, kind="ExternalOutput")
    with tile.TileContext(nc) as tc:
        _emit(tc, x.ap(), wq.ap(), wk.ap(), wv.ap(), bqk.ap(), gbv.ap(),
              gamma.ap(), y.ap())
    nc.compile()
    return nc


def make_in_maps(inputs):
    xf = np.ascontiguousarray(
        np.asarray(inputs["x"], dtype=np.float32).reshape(B, C, N))
    # WSCALE pre-scaling keeps the fp8e4 weights out of the subnormal
    # range; the score-side 256x is undone by the exp activation scale,
    # the V-side 16x by dividing gamma.
    wq = np.ascontiguousarray(
        np.asarray(inputs["wq"], dtype=np.float32) * WSCALE)
    wk = np.ascontiguousarray(
        np.asarray(inputs["wk"], dtype=np.float32) * WSCALE)
    wv = np.ascontiguousarray(
        np.asarray(inputs["wv"], dtype=np.float32) * WSCALE)
    bqk = np.concatenate([
        np.asarray(inputs["bq"], dtype=np.float32),
        np.asarray(inputs["bk"], dtype=np.float32),
    ]).reshape(128, 1) * WSCALE
    gamma_v = float(np.asarray(inputs["gamma"], dtype=np.float32).reshape(()))
    # v-bias contributes exactly gamma*bv to y (softmax rows sum to 1)
    gbv = np.ascontiguousarray(
        (gamma_v * np.asarray(inputs["bv"], dtype=np.float32))
        .reshape(4, 128).T)
    gamma = np.full((1, 1), gamma_v / WSCALE, dtype=np.float32)
    in_maps = []
    for i in range(NCORES):
        b, h = divmod(i, 2)
        xr = np.roll(xf[b], -h * NH, axis=1) if h else xf[b]
        in_maps.append({
            "x": np.ascontiguousarray(xr), "wq": wq, "wk": wk, "wv": wv,
            "bqk": bqk, "gbv": gbv, "gamma": gamma,
        })
    return in_maps


_NC = None


def _get_nc():
    global _NC
    if _NC is None:
        _NC = build_nc()
    return _NC


def kernel(**inputs):
    nc = _get_nc()
    in_maps = make_in_maps(inputs)
    res = bass_utils.run_bass_kernel_spmd(nc, in_maps, core_ids=list(range(NCORES)))
    yf = np.empty((B, C, N), dtype=np.float32)
    for i in range(NCORES):
        b, h = divmod(i, 2)
        yf[b][:, h * NH:(h + 1) * NH] = res.results[i]["y"]
    return yf.reshape(B, C, W, H)


# revision 14
# speedup vs baseline: 1.0409x; 1.0409x over previous
"""Channel-attention kernel for Trainium2, SPMD across 8 NeuronCores.

Problem: x:[4,512,64,64] f32; q = wq@x+bq, k = wk@x+bk (Cq=64), v = wv@x+bv;
scores = q^T k -> [B,4096,4096]; attn = softmax(scores, -1);
out = v @ attn^T; y = gamma*out + x.

Sharding: 8 shards = 4 batches x 2 query-halves. Each core gets its batch's
x pre-rotated along the pixel axis so its 2048 queries sit in columns 0:2048
(softmax/AV are permutation-invariant over keys, so rotating keys/values is
harmless). This keeps the SPMD program identical on every core.

Startup is HBM-read-roofline bound (x is 8MB f32 and must be read once), so
the load is organized to start compute per pixel-half:
  - sync ring stages rows 0:256 as f32 (also serving as the exact-fp32
    residual source for channel tiles 0/1); gpsimd casting DMAs load rows
    256:512 straight to fp8e4. The scalar engine casts the staged half-0
    chunks to fp8, DVE casts half-1, so no DMA ring ever feeds another ring.
  - q/k bias-adds write both duplicated partition halves of the score
    operands directly (no SBUF->SBUF duplication DMAs), so half-0 score
    pairs + their exps start while the second half of x is still in flight.

Per-core pipeline (fp8 DoubleRow on the PE wherever K>=256):
  - QK projection in fp8 DR straight from xp (wq/wk/bq/bk pre-scaled x16
    on the host to keep fp8e4 weights out of the subnormal range; the 256x
    score scale is removed for free by the exp activation's scale).
  - V projection computed transposed (vT[m,c] = x^T wvT) in fp8 DR. The v
    bias is NOT added: it contributes exactly gamma*bv to y (softmax rows
    sum to 1), folded into the output add via a host-precomputed gbv.
  - ScoresT[m,n] = k^T q as K=64 pairs on disjoint row-groups (concurrent);
    exp on the scalar engine -> fp8e5 (scale=1/256, bias=-4).
  - Softmax denominators d[n] via an all-ones [128,2,128] DR stationary:
    each dps matmul broadcasts d[n] to all 128 partitions, directly
    usable in the [c,n] layout (reciprocal_approx_fast + gamma on DVE).
  - AV in the residual layout: out[c,n] = sum_m vT[m,c] e[m,n]
    (lhsT = vP slice, rhs = expP). No transposes anywhere.
  - y = av*(gamma/d) + gbv + x as tensor_mul + one fused
    scalar_tensor_tensor; one wide output DMA per group (per-tile for the
    last group to shorten the drain).

Precision: fp8 Q/K/V projections are well within the 2e-2 gate (errors
average out across the 4096-key softmax support and 512-channel
contractions); the residual path keeps x in exact fp32 end to end.
"""

import numpy as np

import concourse.bass as bass
import concourse.bacc as bacc
import concourse.mybir as mybir
import concourse.tile as tile
from concourse import bass_utils

B, C, W, H = 4, 512, 64, 64
N = W * H          # 4096 pixels
CQ = 64            # query/key channels
NH = N // 2        # 2048 queries per core
NCORES = 8
F32 = mybir.dt.float32
BF16 = mybir.dt.bfloat16
FP8E4 = mybir.dt.float8e4
FP8E5 = mybir.dt.float8e5
DR = mybir.MatmulPerfMode.DoubleRow
ALU = mybir.AluOpType
VPAD = 528   # fp8 vT pair stride, %16 == 0
AF = mybir.ActivationFunctionType
WSCALE = 16.0          # host pre-scale on wq/wk/wv (and q/k biases)
ESCALE = 1.0 / (WSCALE * WSCALE)   # undone inside the exp activation

N_MT = N // 128    # 32 key tiles
N_G = NH // 512    # 4 query groups per core
NJ = N_MT // 2     # 16 fp8 pair tiles


def _emit(tc, x, wqkT, wvT, cst, y):
    nc = tc.nc

    with (
        tc.tile_pool(name="const", bufs=1) as const,
        tc.tile_pool(name="data", bufs=1) as data,
        tc.tile_pool(name="stg", bufs=2) as stg,
        tc.tile_pool(name="outp", bufs=2) as outp,
        tc.tile_pool(name="ps_sc", bufs=2, space="PSUM") as ps_sc,
        tc.tile_pool(name="ps_av", bufs=2, space="PSUM") as ps_av,
        tc.tile_pool(name="ps_d", bufs=2, space="PSUM") as ps_d,
    ):
        # ---- sync ring: consts, transposed q/k weights, x staging ------
        cst_s = const.tile([128, 8], F32, tag="cst")
        nc.sync.dma_start(cst_s[:], cst)
        wqk_f = stg.tile([128, 1024], F32, tag="wqkf")
        nc.sync.dma_start(wqk_f[:, 0:512], wqkT[:, 0:512])
        nc.sync.dma_start(wqk_f[:, 512:1024], wqkT[:, 512:1024])
        gsc_s = cst_s[:, 1:2]     # gamma / WSCALE, broadcast
        nbias = cst_s[:, 6:7]     # -4.0

        # ---- gpsimd ring: rows 256:512 fp8 casts (h0), wv, casts (h1) --
        xp = [data.tile([128, 2 * N], FP8E4, tag=f"xp{pc}", name=f"xp{pc}")
              for pc in range(2)]
        xst = [data.tile([128, NH], F32, tag=f"xs{i}", name=f"xs{i}")
               for i in range(4)]
        wvT_f = stg.tile([128, 2048], F32, tag="wvtf")

        def stage_half(hh):
            for r in range(2):
                xs = xst[hh * 2 + r]
                for q in range(2):
                    nc.sync.dma_start(
                        xs[:, q * 1024:(q + 1) * 1024],
                        x[r * 128:(r + 1) * 128,
                          hh * NH + q * 1024:hh * NH + (q + 1) * 1024])
                for q in range(2):
                    nc.gpsimd.dma_start(
                        xp[1][:, (r % 2) * N + hh * NH + q * 1024:
                               (r % 2) * N + hh * NH + (q + 1) * 1024],
                        x[(r + 2) * 128:(r + 3) * 128,
                          hh * NH + q * 1024:hh * NH + (q + 1) * 1024])

        stage_half(0)
        for q in range(2):
            nc.gpsimd.dma_start(wvT_f[:, q * 1024:(q + 1) * 1024],
                                wvT[:, q * 1024:(q + 1) * 1024])
        stage_half(1)

        # x residual rows 256:512 (channel tiles 2/3); tiles 0/1 reuse xst
        xr23 = [data.tile([128, 2048], F32, tag=f"xr{ci}", name=f"xr2{ci}")
                for ci in range(2)]
        for ci in range(2):
            for q in range(2):
                nc.sync.dma_start(
                    xr23[ci][:, q * 1024:(q + 1) * 1024],
                    x[(2 + ci) * 128:(3 + ci) * 128,
                      q * 1024:(q + 1) * 1024])

        def xres(g, ct):
            src = (xst[ct] if ct < 2 else xr23[ct - 2])
            return src[:, g * 512:(g + 1) * 512]

        # ---- scalar engine casts staged half-0 to fp8 ------------------
        def cast_half(eng, hh):
            for r in range(2):
                xs = xst[hh * 2 + r]
                for q in range(2):
                    dst = xp[0][:, (r % 2) * N + hh * NH + q * 1024:
                                (r % 2) * N + hh * NH + (q + 1) * 1024]
                    src = xs[:, q * 1024:(q + 1) * 1024]
                    if eng == "act":
                        nc.scalar.activation(dst, src, AF.Copy)
                    else:
                        nc.vector.tensor_copy(dst, src)

        cast_half("act", 0)

        # ---- constants / weight prep: plain DVE casts (host transposed)
        onesDR = const.tile([128, 256], FP8E4, tag="onesDR")
        nc.gpsimd.memset(onesDR[:], 1.0)

        wqq = const.tile([128, 512], FP8E4, tag="wqq")
        nc.vector.tensor_copy(wqq[:], wqk_f[:, 0:512])
        wkk = const.tile([128, 512], FP8E4, tag="wkk")
        nc.vector.tensor_copy(wkk[:], wqk_f[:, 512:1024])

        # ---- QK + V projection, chasing the DMAs; q/k written directly
        #      into both partition halves of their score operands ---------
        q2 = data.tile([128, NH], BF16, tag="q2")
        k2 = data.tile([128, N], BF16, tag="k2")
        vP = [data.tile([128, 2 * VPAD], FP8E4, tag=f"vP{j}", name=f"vP{j}")
              for j in range(NJ)]
        wvTp = [const.tile([128, 1024], FP8E4, tag=f"wvTp{pc}",
                           name=f"wvTp{pc}") for pc in range(2)]

        def score_pair(expP_list, g, j):
            mA, mB = 2 * j, 2 * j + 1
            q_lo = q2[0:CQ, g * 512:(g + 1) * 512]
            q_hi = q2[CQ:128, g * 512:(g + 1) * 512]
            ps = ps_sc.tile([128, 1024], F32, tag="sc", name=f"ps{g}_{j}")
            nc.tensor.matmul(
                ps[:, 0:512], k2[0:CQ, mA * 128:(mA + 1) * 128], q_lo,
                start=True, stop=True,
            )
            nc.tensor.matmul(
                ps[:, 512:1024],
                k2[CQ:128, mB * 128:(mB + 1) * 128], q_hi,
                start=True, stop=True,
            )
            nc.scalar.activation(expP_list[j][:], ps[:], AF.Exp,
                                 bias=nbias, scale=ESCALE)

        def qk_half(hh):
            for gg in range(4):
                g = hh * 4 + gg
                cols = slice(g * 512, (g + 1) * 512)
                targets = [(wkk, k2[:, cols], cst_s[:, 7:8])]
                if hh == 0:
                    targets.append((wqq, q2[:, cols], cst_s[:, 0:1]))
                for wt, dst, bias in targets:
                    ps = ps_av.tile([128, 512], F32, tag="av",
                                    name=f"qk{g}_{dst.tensor.name}")
                    for pc in range(2):
                        nc.tensor.matmul(
                            ps[:],
                            wt[:, pc * 256:(pc + 1) * 256].rearrange(
                                "p (i n) -> p i n", i=2),
                            xp[pc][:].rearrange("p (i n) -> p i n", i=2)[
                                :, :, g * 512:(g + 1) * 512],
                            start=(pc == 0), stop=(pc == 1), perf_mode=DR,
                        )
                    nc.vector.tensor_scalar_add(dst, ps[:], bias)

        def vproj_half(hh):
            for mt in range(hh * 16, hh * 16 + 16):
                ps = ps_d.tile([128, C], F32, tag="d", name=f"vps{mt}")
                for pc in range(2):
                    nc.tensor.matmul(
                        ps[:],
                        xp[pc][:].rearrange("p (i n) -> p i n", i=2)[
                            :, :, mt * 128:(mt + 1) * 128],
                        wvTp[pc][:].rearrange("p (i n) -> p i n", i=2),
                        start=(pc == 0), stop=(pc == 1), perf_mode=DR,
                    )
                j, half = divmod(mt, 2)
                nc.vector.tensor_copy(vP[j][:, half * VPAD:half * VPAD + C],
                                      ps[:])

        expP = [data.tile([128, 1024], FP8E5, tag=f"expP{j}",
                          name=f"expP{j}_0", bufs=2) for j in range(NJ)]

        qk_half(0)
        for j in range(NJ // 2):
            score_pair(expP, 0, j)

        # wvT fp8 casts (host transposed; lands mid-load on gpsimd ring)
        for pc in range(2):
            nc.vector.tensor_copy(wvTp[pc][:],
                                  wvT_f[:, pc * 1024:(pc + 1) * 1024])
        vproj_half(0)

        cast_half("dve", 1)
        qk_half(1)
        for j in range(NJ // 2, NJ):
            score_pair(expP, 0, j)
        vproj_half(1)

        # ---- attention -------------------------------------------------
        def alloc_expP(g):
            return [data.tile([128, 1024], FP8E5, tag=f"expP{j}",
                              name=f"expP{j}_{g}", bufs=2)
                    for j in range(NJ)]

        yv = y.rearrange("(ct p) n -> p ct n", ct=4)
        for g in range(N_G):
            nxt = alloc_expP(g + 1) if g + 1 < N_G else None

            # denominators broadcast to all partitions via all-ones DR
            dsum = ps_d.tile([128, 512], F32, tag="d", name=f"dsum{g}")
            ones_ap = onesDR[:].rearrange("p (i n) -> p i n", i=2)
            for j in range(NJ):
                nc.tensor.matmul(
                    dsum[:], ones_ap,
                    expP[j][:].rearrange("p (i n) -> p i n", i=2),
                    start=(j == 0), stop=(j == NJ - 1), perf_mode=DR,
                )
            dinv = data.tile([128, 512], F32, tag="dinv", bufs=2,
                             name=f"dinv{g}")
            nc.vector.reciprocal_approx_fast(dinv[:], dsum[:])
            nc.vector.tensor_scalar_mul(dinv[:], dinv[:], gsc_s)

            yow = outp.tile([128, 2048], F32, tag="yo", name=f"yo{g}")
            for ct in range(4):
                # interleave next group's scores to keep ACT fed
                if nxt is not None:
                    for j in range(ct * 4, ct * 4 + 4):
                        score_pair(nxt, g + 1, j)
                av = ps_av.tile([128, 512], F32, tag="av",
                                name=f"av{g}_{ct}")
                for j in range(NJ):
                    nc.tensor.matmul(
                        av[:],
                        vP[j][:].rearrange("p (i n) -> p i n", i=2)[
                            :, :, ct * 128:(ct + 1) * 128],
                        expP[j][:].rearrange("p (i n) -> p i n", i=2),
                        start=(j == 0), stop=(j == NJ - 1), perf_mode=DR,
                    )
                t0 = outp.tile([128, 512], F32, tag="ysc")
                nc.vector.tensor_mul(t0[:], av[:], dinv[:])
                yslice = yow[:, ct * 512:(ct + 1) * 512]
                nc.vector.scalar_tensor_tensor(
                    yslice, t0[:], cst_s[:, 2 + ct:3 + ct], xres(g, ct),
                    ALU.add, ALU.add)
                if g == N_G - 1:
                    # last group: drain per tile
                    nc.sync.dma_start(
                        y[ct * 128:(ct + 1) * 128, g * 512:(g + 1) * 512],
                        yslice)
            if g < N_G - 1:
                nc.sync.dma_start(
                    yv[:, :, g * 512:(g + 1) * 512],
                    yow[:].rearrange("p (ct n) -> p ct n", ct=4))
            expP = nxt


def build_nc():
    nc = bacc.Bacc("TRN2", target_bir_lowering=False, debug=False,
                   num_devices=NCORES)
    x = nc.dram_tensor("x", [C, N], F32, kind="ExternalInput")
    wqkT = nc.dram_tensor("wqkT", [128, 1024], F32, kind="ExternalInput")
    wvT = nc.dram_tensor("wvT", [128, 2048], F32, kind="ExternalInput")
    cst = nc.dram_tensor("cst", [128, 8], F32, kind="ExternalInput")
    y = nc.dram_tensor("y", [C, NH], F32, kind="ExternalOutput")
    with tile.TileContext(nc) as tc:
        _emit(tc, x.ap(), wqkT.ap(), wvT.ap(), cst.ap(), y.ap())
    nc.compile()
    return nc


def make_in_maps(inputs):
    xf = np.ascontiguousarray(
        np.asarray(inputs["x"], dtype=np.float32).reshape(B, C, N))
    # WSCALE pre-scaling keeps the fp8e4 weights out of the subnormal
    # range; the score-side 256x is undone by the exp activation scale,
    # the V-side 16x by dividing gamma. Weights are host-transposed into
    # the DoubleRow stationary layout [p, pc*F + i*(F//2) + m] =
    # w[m, pc*256 + i*128 + p], with q (and k) duplicated across both
    # output column halves so each lands in both score-operand halves.
    wq = np.asarray(inputs["wq"], dtype=np.float32) * WSCALE
    wk = np.asarray(inputs["wk"], dtype=np.float32) * WSCALE
    wv = np.asarray(inputs["wv"], dtype=np.float32) * WSCALE

    def dr_layout(w):
        # w: [M, 512] -> [128, 2, 2, M]: [p, pc, i, m] = w[m, pc*256+i*128+p]
        return np.ascontiguousarray(
            w.T.reshape(2, 2, 128, w.shape[0]).transpose(2, 0, 1, 3)
            .reshape(128, -1))

    wqkT = np.concatenate(
        [dr_layout(np.concatenate([wq, wq], axis=0)),
         dr_layout(np.concatenate([wk, wk], axis=0))], axis=1)
    wvT = dr_layout(wv)
    gamma_v = float(np.asarray(inputs["gamma"], dtype=np.float32).reshape(()))
    cst = np.zeros((128, 8), dtype=np.float32)
    bq2 = np.concatenate([np.asarray(inputs["bq"], dtype=np.float32)] * 2)
    bk2 = np.concatenate([np.asarray(inputs["bk"], dtype=np.float32)] * 2)
    cst[:, 0] = bq2 * WSCALE
    cst[:, 7] = bk2 * WSCALE
    cst[:, 1] = gamma_v / WSCALE
    # v-bias contributes exactly gamma*bv to y (softmax rows sum to 1)
    cst[:, 2:6] = (gamma_v * np.asarray(inputs["bv"], dtype=np.float32)
                   ).reshape(4, 128).T
    cst[:, 6] = -4.0
    in_maps = []
    for i in range(NCORES):
        b, h = divmod(i, 2)
        xr = np.roll(xf[b], -h * NH, axis=1) if h else xf[b]
        in_maps.append({
            "x": np.ascontiguousarray(xr), "wqkT": wqkT, "wvT": wvT,
            "cst": cst,
        })
    return in_maps


_NC = None


def _get_nc():
    global _NC
    if _NC is None:
        _NC = build_nc()
    return _NC


def kernel(**inputs):
    nc = _get_nc()
    in_maps = make_in_maps(inputs)
    res = bass_utils.run_bass_kernel_spmd(nc, in_maps, core_ids=list(range(NCORES)))
    yf = np.empty((B, C, N), dtype=np.float32)
    for i in range(NCORES):
        b, h = divmod(i, 2)
        yf[b][:, h * NH:(h + 1) * NH] = res.results[i]["y"]
    return yf.reshape(B, C, W, H)


# revision 15
# speedup vs baseline: 1.0760x; 1.0338x over previous
"""Channel-attention kernel for Trainium2, SPMD across 8 NeuronCores.

Problem: x:[4,512,64,64] f32; q = wq@x+bq, k = wk@x+bk (Cq=64), v = wv@x+bv;
scores = q^T k -> [B,4096,4096]; attn = softmax(scores, -1);
out = v @ attn^T; y = gamma*out + x.

Sharding: 8 shards = 4 batches x 2 query-halves. Each core gets its batch's
x pre-rotated along the pixel axis so its 2048 queries sit in columns 0:2048
(softmax/AV are permutation-invariant over keys, so rotating keys/values is
harmless). This keeps the SPMD program identical on every core.

Startup is HBM-read-roofline bound (x is 8MB f32 and must be read once), so
the load is organized to start compute per pixel-half:
  - sync ring stages rows 0:256 as f32 (also serving as the exact-fp32
    residual source for channel tiles 0/1); gpsimd casting DMAs load rows
    256:512 straight to fp8e4. The scalar engine casts the staged half-0
    chunks to fp8, DVE casts half-1, so no DMA ring ever feeds another ring.
  - q/k bias-adds write both duplicated partition halves of the score
    operands directly (no SBUF->SBUF duplication DMAs), so half-0 score
    pairs + their exps start while the second half of x is still in flight.

Per-core pipeline (fp8 DoubleRow on the PE wherever K>=256):
  - QK projection in fp8 DR straight from xp (wq/wk/bq/bk pre-scaled x16
    on the host to keep fp8e4 weights out of the subnormal range; the 256x
    score scale is removed for free by the exp activation's scale).
  - V projection computed transposed (vT[m,c] = x^T wvT) in fp8 DR. The v
    bias is NOT added: it contributes exactly gamma*bv to y (softmax rows
    sum to 1), folded into the output add via a host-precomputed gbv.
  - ScoresT[m,n] = k^T q as K=64 pairs on disjoint row-groups (concurrent);
    exp on the scalar engine -> fp8e5 (scale=1/256, bias=-4).
  - Softmax denominators d[n] via an all-ones [128,2,128] DR stationary:
    each dps matmul broadcasts d[n] to all 128 partitions, directly
    usable in the [c,n] layout (reciprocal_approx_fast + gamma on DVE).
  - AV in the residual layout: out[c,n] = sum_m vT[m,c] e[m,n]
    (lhsT = vP slice, rhs = expP). No transposes anywhere.
  - y = av*(gamma/d) + gbv + x as tensor_mul + one fused
    scalar_tensor_tensor; one wide output DMA per group (per-tile for the
    last group to shorten the drain).

Precision: fp8 Q/K/V projections are well within the 2e-2 gate (errors
average out across the 4096-key softmax support and 512-channel
contractions); the residual path keeps x in exact fp32 end to end.
"""

import numpy as np

import concourse.bass as bass
import concourse.bacc as bacc
import concourse.mybir as mybir
import concourse.tile as tile
from concourse import bass_utils

B, C, W, H = 4, 512, 64, 64
N = W * H          # 4096 pixels
CQ = 64            # query/key channels
NH = N // 2        # 2048 queries per core
NCORES = 8
F32 = mybir.dt.float32
BF16 = mybir.dt.bfloat16
FP8E4 = mybir.dt.float8e4
FP8E5 = mybir.dt.float8e5
DR = mybir.MatmulPerfMode.DoubleRow
ALU = mybir.AluOpType
VPAD = 528   # fp8 vT pair stride, %16 == 0
AF = mybir.ActivationFunctionType
WSCALE = 16.0          # host pre-scale on wq/wk/wv (and q/k biases)
ESCALE = 1.0 / (WSCALE * WSCALE)   # undone inside the exp activation

N_MT = N // 128    # 32 key tiles
N_G = NH // 512    # 4 query groups per core
NJ = N_MT // 2     # 16 fp8 pair tiles


def _emit(tc, x, wqkT, wvT, cst, y):
    nc = tc.nc

    with (
        tc.tile_pool(name="const", bufs=1) as const,
        tc.tile_pool(name="data", bufs=1) as data,
        tc.tile_pool(name="stg", bufs=2) as stg,
        tc.tile_pool(name="outp", bufs=2) as outp,
        tc.tile_pool(name="ps_sc", bufs=2, space="PSUM") as ps_sc,
        tc.tile_pool(name="ps_av", bufs=2, space="PSUM") as ps_av,
        tc.tile_pool(name="ps_d", bufs=2, space="PSUM") as ps_d,
    ):
        # ---- sync ring: consts, transposed q/k weights, x staging ------
        cst_s = const.tile([128, 8], F32, tag="cst")
        nc.sync.dma_start(cst_s[:], cst)
        wqk_f = stg.tile([128, 1024], BF16, tag="wqkf")
        nc.sync.dma_start(wqk_f[:, 0:512], wqkT[:, 0:512])
        nc.sync.dma_start(wqk_f[:, 512:1024], wqkT[:, 512:1024])
        gsc_s = cst_s[:, 1:2]     # gamma / WSCALE, broadcast
        nbias = cst_s[:, 6:7]     # -4.0

        # ---- gpsimd ring: rows 256:512 fp8 casts (h0), wv, casts (h1) --
        xp = [data.tile([128, 2 * N], FP8E4, tag=f"xp{pc}", name=f"xp{pc}")
              for pc in range(2)]
        xst = [data.tile([128, NH], F32, tag=f"xs{i}", name=f"xs{i}")
               for i in range(4)]
        wvT_f = stg.tile([128, 2048], BF16, tag="wvtf")

        def stage_half(hh):
            # q-major so QK groups unblock after the first column chunks
            for q in range(2):
                for r in range(2):
                    xs = xst[hh * 2 + r]
                    nc.sync.dma_start(
                        xs[:, q * 1024:(q + 1) * 1024],
                        x[r * 128:(r + 1) * 128,
                          hh * NH + q * 1024:hh * NH + (q + 1) * 1024])
                for r in range(2):
                    nc.gpsimd.dma_start(
                        xp[1][:, (r % 2) * N + hh * NH + q * 1024:
                               (r % 2) * N + hh * NH + (q + 1) * 1024],
                        x[(r + 2) * 128:(r + 3) * 128,
                          hh * NH + q * 1024:hh * NH + (q + 1) * 1024])

        stage_half(0)
        for q in range(2):
            nc.gpsimd.dma_start(wvT_f[:, q * 1024:(q + 1) * 1024],
                                wvT[:, q * 1024:(q + 1) * 1024])
        stage_half(1)

        # x residual rows 256:512 (channel tiles 2/3); tiles 0/1 reuse xst
        xr23 = [data.tile([128, 2048], F32, tag=f"xr{ci}", name=f"xr2{ci}")
                for ci in range(2)]
        for ci in range(2):
            for q in range(2):
                nc.sync.dma_start(
                    xr23[ci][:, q * 1024:(q + 1) * 1024],
                    x[(2 + ci) * 128:(3 + ci) * 128,
                      q * 1024:(q + 1) * 1024])

        def xres(g, ct):
            src = (xst[ct] if ct < 2 else xr23[ct - 2])
            return src[:, g * 512:(g + 1) * 512]

        # ---- scalar engine casts staged half-0 to fp8 ------------------
        def cast_half(eng, hh):
            for q in range(2):
                for r in range(2):
                    xs = xst[hh * 2 + r]
                    dst = xp[0][:, (r % 2) * N + hh * NH + q * 1024:
                                (r % 2) * N + hh * NH + (q + 1) * 1024]
                    src = xs[:, q * 1024:(q + 1) * 1024]
                    if eng == "act":
                        nc.scalar.activation(dst, src, AF.Copy)
                    else:
                        nc.vector.tensor_copy(dst, src)

        cast_half("act", 0)

        # ---- constants / weight prep: plain DVE casts (host transposed)
        onesDR = const.tile([128, 256], FP8E4, tag="onesDR")
        nc.gpsimd.memset(onesDR[:], 1.0)

        wqq = const.tile([128, 512], FP8E4, tag="wqq")
        nc.vector.tensor_copy(wqq[:], wqk_f[:, 0:512])
        wkk = const.tile([128, 512], FP8E4, tag="wkk")
        nc.vector.tensor_copy(wkk[:], wqk_f[:, 512:1024])

        # ---- QK + V projection, chasing the DMAs; q/k written directly
        #      into both partition halves of their score operands ---------
        q2 = data.tile([128, NH], BF16, tag="q2")
        k2 = data.tile([128, N], BF16, tag="k2")
        vP = [data.tile([128, 2 * VPAD], FP8E4, tag=f"vP{j}", name=f"vP{j}")
              for j in range(NJ)]
        wvTp = [const.tile([128, 1024], FP8E4, tag=f"wvTp{pc}",
                           name=f"wvTp{pc}") for pc in range(2)]

        def score_pair(expP_list, g, j):
            mA, mB = 2 * j, 2 * j + 1
            q_lo = q2[0:CQ, g * 512:(g + 1) * 512]
            q_hi = q2[CQ:128, g * 512:(g + 1) * 512]
            ps = ps_sc.tile([128, 1024], F32, tag="sc", name=f"ps{g}_{j}")
            nc.tensor.matmul(
                ps[:, 0:512], k2[0:CQ, mA * 128:(mA + 1) * 128], q_lo,
                start=True, stop=True,
            )
            nc.tensor.matmul(
                ps[:, 512:1024],
                k2[CQ:128, mB * 128:(mB + 1) * 128], q_hi,
                start=True, stop=True,
            )
            nc.scalar.activation(expP_list[j][:], ps[:], AF.Exp,
                                 bias=nbias, scale=ESCALE)

        def qk_half(hh):
            for gg in range(4):
                g = hh * 4 + gg
                cols = slice(g * 512, (g + 1) * 512)
                targets = [(wkk, k2[:, cols], cst_s[:, 7:8])]
                if hh == 0:
                    targets.append((wqq, q2[:, cols], cst_s[:, 0:1]))
                for wt, dst, bias in targets:
                    ps = ps_av.tile([128, 512], F32, tag="av",
                                    name=f"qk{g}_{dst.tensor.name}")
                    for pc in range(2):
                        nc.tensor.matmul(
                            ps[:],
                            wt[:, pc * 256:(pc + 1) * 256].rearrange(
                                "p (i n) -> p i n", i=2),
                            xp[pc][:].rearrange("p (i n) -> p i n", i=2)[
                                :, :, g * 512:(g + 1) * 512],
                            start=(pc == 0), stop=(pc == 1), perf_mode=DR,
                        )
                    nc.vector.tensor_scalar_add(dst, ps[:], bias)

        def vproj_half(hh):
            for mt in range(hh * 16, hh * 16 + 16):
                ps = ps_d.tile([128, C], F32, tag="d", name=f"vps{mt}")
                for pc in range(2):
                    nc.tensor.matmul(
                        ps[:],
                        xp[pc][:].rearrange("p (i n) -> p i n", i=2)[
                            :, :, mt * 128:(mt + 1) * 128],
                        wvTp[pc][:].rearrange("p (i n) -> p i n", i=2),
                        start=(pc == 0), stop=(pc == 1), perf_mode=DR,
                    )
                j, half = divmod(mt, 2)
                nc.vector.tensor_copy(vP[j][:, half * VPAD:half * VPAD + C],
                                      ps[:])

        expP = [data.tile([128, 1024], FP8E5, tag=f"expP{j}",
                          name=f"expP{j}_0", bufs=2) for j in range(NJ)]

        qk_half(0)
        for j in range(NJ // 2):
            score_pair(expP, 0, j)

        # wvT fp8 casts (host transposed; lands mid-load on gpsimd ring)
        for pc in range(2):
            nc.vector.tensor_copy(wvTp[pc][:],
                                  wvT_f[:, pc * 1024:(pc + 1) * 1024])
        vproj_half(0)

        cast_half("dve", 1)
        qk_half(1)
        for j in range(NJ // 2, NJ):
            score_pair(expP, 0, j)
        vproj_half(1)

        # ---- attention -------------------------------------------------
        def alloc_expP(g):
            return [data.tile([128, 1024], FP8E5, tag=f"expP{j}",
                              name=f"expP{j}_{g}", bufs=2)
                    for j in range(NJ)]

        yv = y.rearrange("(ct p) n -> p ct n", ct=4)
        for g in range(N_G):
            nxt = alloc_expP(g + 1) if g + 1 < N_G else None

            # denominators broadcast to all partitions via all-ones DR
            dsum = ps_d.tile([128, 512], F32, tag="d", name=f"dsum{g}")
            ones_ap = onesDR[:].rearrange("p (i n) -> p i n", i=2)
            for j in range(NJ):
                nc.tensor.matmul(
                    dsum[:], ones_ap,
                    expP[j][:].rearrange("p (i n) -> p i n", i=2),
                    start=(j == 0), stop=(j == NJ - 1), perf_mode=DR,
                )
            dinv = data.tile([128, 512], F32, tag="dinv", bufs=2,
                             name=f"dinv{g}")
            nc.vector.reciprocal_approx_fast(dinv[:], dsum[:])
            nc.vector.tensor_scalar_mul(dinv[:], dinv[:], gsc_s)

            yow = outp.tile([128, 2048], F32, tag="yo", name=f"yo{g}")
            for ct in range(4):
                # interleave next group's scores to keep ACT fed
                if nxt is not None:
                    for j in range(ct * 4, ct * 4 + 4):
                        score_pair(nxt, g + 1, j)
                av = ps_av.tile([128, 512], F32, tag="av",
                                name=f"av{g}_{ct}")
                for j in range(NJ):
                    nc.tensor.matmul(
                        av[:],
                        vP[j][:].rearrange("p (i n) -> p i n", i=2)[
                            :, :, ct * 128:(ct + 1) * 128],
                        expP[j][:].rearrange("p (i n) -> p i n", i=2),
                        start=(j == 0), stop=(j == NJ - 1), perf_mode=DR,
                    )
                t0 = outp.tile([128, 512], F32, tag="ysc")
                nc.vector.tensor_mul(t0[:], av[:], dinv[:])
                yslice = yow[:, ct * 512:(ct + 1) * 512]
                nc.vector.scalar_tensor_tensor(
                    yslice, t0[:], cst_s[:, 2 + ct:3 + ct], xres(g, ct),
                    ALU.add, ALU.add)
                if g == N_G - 1:
                    # last group: drain per tile
                    nc.sync.dma_start(
                        y[ct * 128:(ct + 1) * 128, g * 512:(g + 1) * 512],
                        yslice)
            if g < N_G - 1:
                nc.sync.dma_start(
                    yv[:, :, g * 512:(g + 1) * 512],
                    yow[:].rearrange("p (ct n) -> p ct n", ct=4))
            expP = nxt


def build_nc():
    nc = bacc.Bacc("TRN2", target_bir_lowering=False, debug=False,
                   num_devices=NCORES)
    x = nc.dram_tensor("x", [C, N], F32, kind="ExternalInput")
    wqkT = nc.dram_tensor("wqkT", [128, 1024], BF16, kind="ExternalInput")
    wvT = nc.dram_tensor("wvT", [128, 2048], BF16, kind="ExternalInput")
    cst = nc.dram_tensor("cst", [128, 8], F32, kind="ExternalInput")
    y = nc.dram_tensor("y", [C, NH], F32, kind="ExternalOutput")
    with tile.TileContext(nc) as tc:
        _emit(tc, x.ap(), wqkT.ap(), wvT.ap(), cst.ap(), y.ap())
    nc.compile()
    return nc


def make_in_maps(inputs):
    xf = np.ascontiguousarray(
        np.asarray(inputs["x"], dtype=np.float32).reshape(B, C, N))
    # WSCALE pre-scaling keeps the fp8e4 weights out of the subnormal
    # range; the score-side 256x is undone by the exp activation scale,
    # the V-side 16x by dividing gamma. Weights are host-transposed into
    # the DoubleRow stationary layout [p, pc*F + i*(F//2) + m] =
    # w[m, pc*256 + i*128 + p], with q (and k) duplicated across both
    # output column halves so each lands in both score-operand halves.
    wq = np.asarray(inputs["wq"], dtype=np.float32) * WSCALE
    wk = np.asarray(inputs["wk"], dtype=np.float32) * WSCALE
    wv = np.asarray(inputs["wv"], dtype=np.float32) * WSCALE

    def dr_layout(w):
        # w: [M, 512] -> [128, 2, 2, M]: [p, pc, i, m] = w[m, pc*256+i*128+p]
        return np.ascontiguousarray(
            w.T.reshape(2, 2, 128, w.shape[0]).transpose(2, 0, 1, 3)
            .reshape(128, -1))

    import ml_dtypes
    wqkT = np.concatenate(
        [dr_layout(np.concatenate([wq, wq], axis=0)),
         dr_layout(np.concatenate([wk, wk], axis=0))], axis=1
    ).astype(ml_dtypes.bfloat16)
    wvT = dr_layout(wv).astype(ml_dtypes.bfloat16)
    gamma_v = float(np.asarray(inputs["gamma"], dtype=np.float32).reshape(()))
    cst = np.zeros((128, 8), dtype=np.float32)
    bq2 = np.concatenate([np.asarray(inputs["bq"], dtype=np.float32)] * 2)
    bk2 = np.concatenate([np.asarray(inputs["bk"], dtype=np.float32)] * 2)
    cst[:, 0] = bq2 * WSCALE
    cst[:, 7] = bk2 * WSCALE
    cst[:, 1] = gamma_v / WSCALE
    # v-bias contributes exactly gamma*bv to y (softmax rows sum to 1)
    cst[:, 2:6] = (gamma_v * np.asarray(inputs["bv"], dtype=np.float32)
                   ).reshape(4, 128).T
    cst[:, 6] = -4.0
    in_maps = []
    for i in range(NCORES):
        b, h = divmod(i, 2)
        xr = np.roll(xf[b], -h * NH, axis=1) if h else xf[b]
        in_maps.append({
            "x": np.ascontiguousarray(xr), "wqkT": wqkT, "wvT": wvT,
            "cst": cst,
        })
    return in_maps


_NC = None


def _get_nc():
    global _NC
    if _NC is None:
        _NC = build_nc()
    return _NC


def kernel(**inputs):
    nc = _get_nc()
    in_maps = make_in_maps(inputs)
    res = bass_utils.run_bass_kernel_spmd(nc, in_maps, core_ids=list(range(NCORES)))
    yf = np.empty((B, C, N), dtype=np.float32)
    for i in range(NCORES):
        b, h = divmod(i, 2)
        yf[b][:, h * NH:(h + 1) * NH] = res.results[i]["y"]
    return yf.reshape(B, C, W, H)


# revision 16
# speedup vs baseline: 1.0850x; 1.0084x over previous
"""Channel-attention kernel for Trainium2, SPMD across 8 NeuronCores.

Problem: x:[4,512,64,64] f32; q = wq@x+bq, k = wk@x+bk (Cq=64), v = wv@x+bv;
scores = q^T k -> [B,4096,4096]; attn = softmax(scores, -1);
out = v @ attn^T; y = gamma*out + x.

Sharding: 8 shards = 4 batches x 2 query-halves. Each core gets its batch's
x pre-rotated along the pixel axis so its 2048 queries sit in columns 0:2048
(softmax/AV are permutation-invariant over keys, so rotating keys/values is
harmless). This keeps the SPMD program identical on every core.

Startup is HBM-read-roofline bound (x is 8MB f32 and must be read once), so
the load is organized to start compute per pixel-half:
  - sync ring stages rows 0:256 as f32 (also serving as the exact-fp32
    residual source for channel tiles 0/1); gpsimd casting DMAs load rows
    256:512 straight to fp8e4. The scalar engine casts the staged half-0
    chunks to fp8, DVE casts half-1, so no DMA ring ever feeds another ring.
  - q/k bias-adds write both duplicated partition halves of the score
    operands directly (no SBUF->SBUF duplication DMAs), so half-0 score
    pairs + their exps start while the second half of x is still in flight.

Per-core pipeline (fp8 DoubleRow on the PE wherever K>=256):
  - QK projection in fp8 DR straight from xp (wq/wk/bq/bk pre-scaled x16
    on the host to keep fp8e4 weights out of the subnormal range; the 256x
    score scale is removed for free by the exp activation's scale).
  - V projection computed transposed (vT[m,c] = x^T wvT) in fp8 DR. The v
    bias is NOT added: it contributes exactly gamma*bv to y (softmax rows
    sum to 1), folded into the output add via a host-precomputed gbv.
  - ScoresT[m,n] = k^T q as K=64 pairs on disjoint row-groups (concurrent);
    exp on the scalar engine -> fp8e5 (scale=1/256, bias=-4).
  - Softmax denominators d[n] via an all-ones [128,2,128] DR stationary:
    each dps matmul broadcasts d[n] to all 128 partitions, directly
    usable in the [c,n] layout (reciprocal_approx_fast + gamma on DVE).
  - AV in the residual layout: out[c,n] = sum_m vT[m,c] e[m,n]
    (lhsT = vP slice, rhs = expP). No transposes anywhere.
  - y = av*(gamma/d) + gbv + x as tensor_mul + one fused
    scalar_tensor_tensor; one wide output DMA per group (per-tile for the
    last group to shorten the drain).

Precision: fp8 Q/K/V projections are well within the 2e-2 gate (errors
average out across the 4096-key softmax support and 512-channel
contractions); the residual path keeps x in exact fp32 end to end.
"""

import numpy as np

import concourse.bass as bass
import concourse.bacc as bacc
import concourse.mybir as mybir
import concourse.tile as tile
from concourse import bass_utils

B, C, W, H = 4, 512, 64, 64
N = W * H          # 4096 pixels
CQ = 64            # query/key channels
NH = N // 2        # 2048 queries per core
NCORES = 8
F32 = mybir.dt.float32
BF16 = mybir.dt.bfloat16
FP8E4 = mybir.dt.float8e4
FP8E5 = mybir.dt.float8e5
DR = mybir.MatmulPerfMode.DoubleRow
ALU = mybir.AluOpType
VPAD = 528   # fp8 vT pair stride, %16 == 0
AF = mybir.ActivationFunctionType
WSCALE = 16.0          # host pre-scale on wq/wk/wv (and q/k biases)
ESCALE = 1.0 / (WSCALE * WSCALE)   # undone inside the exp activation

N_MT = N // 128    # 32 key tiles
N_G = NH // 512    # 4 query groups per core
NJ = N_MT // 2     # 16 fp8 pair tiles


def _emit(tc, x, wqkT, wvT, cst, y):
    nc = tc.nc

    with (
        tc.tile_pool(name="const", bufs=1) as const,
        tc.tile_pool(name="data", bufs=1) as data,
        tc.tile_pool(name="stg", bufs=2) as stg,
        tc.tile_pool(name="outp", bufs=2) as outp,
        tc.tile_pool(name="ps_sc", bufs=2, space="PSUM") as ps_sc,
        tc.tile_pool(name="ps_av", bufs=2, space="PSUM") as ps_av,
        tc.tile_pool(name="ps_d", bufs=2, space="PSUM") as ps_d,
    ):
        # ---- sync ring: consts, transposed q/k weights, x staging ------
        cst_s = const.tile([128, 8], F32, tag="cst")
        nc.sync.dma_start(cst_s[:], cst)
        wqk_f = stg.tile([128, 1024], BF16, tag="wqkf")
        nc.sync.dma_start(wqk_f[:, 0:512], wqkT[:, 0:512])
        nc.sync.dma_start(wqk_f[:, 512:1024], wqkT[:, 512:1024])
        gsc_s = cst_s[:, 1:2]     # gamma / WSCALE, broadcast
        nbias = cst_s[:, 6:7]     # -4.0

        # ---- gpsimd ring: rows 256:512 fp8 casts (h0), wv, casts (h1) --
        xp = [data.tile([128, 2 * N], FP8E4, tag=f"xp{pc}", name=f"xp{pc}")
              for pc in range(2)]
        xst = [data.tile([128, NH], F32, tag=f"xs{i}", name=f"xs{i}")
               for i in range(4)]
        wvT_f = stg.tile([128, 2048], BF16, tag="wvtf")

        def stage_half(hh):
            # q-major so QK groups unblock after the first column chunks
            for q in range(2):
                for r in range(2):
                    xs = xst[hh * 2 + r]
                    nc.sync.dma_start(
                        xs[:, q * 1024:(q + 1) * 1024],
                        x[r * 128:(r + 1) * 128,
                          hh * NH + q * 1024:hh * NH + (q + 1) * 1024])
                for r in range(2):
                    nc.gpsimd.dma_start(
                        xp[1][:, (r % 2) * N + hh * NH + q * 1024:
                               (r % 2) * N + hh * NH + (q + 1) * 1024],
                        x[(r + 2) * 128:(r + 3) * 128,
                          hh * NH + q * 1024:hh * NH + (q + 1) * 1024])

        stage_half(0)
        for q in range(2):
            nc.gpsimd.dma_start(wvT_f[:, q * 1024:(q + 1) * 1024],
                                wvT[:, q * 1024:(q + 1) * 1024])
        stage_half(1)

        # x residual rows 256:512 (channel tiles 2/3); tiles 0/1 reuse xst
        xr23 = [data.tile([128, 2048], F32, tag=f"xr{ci}", name=f"xr2{ci}")
                for ci in range(2)]
        for ci in range(2):
            for q in range(2):
                nc.sync.dma_start(
                    xr23[ci][:, q * 1024:(q + 1) * 1024],
                    x[(2 + ci) * 128:(3 + ci) * 128,
                      q * 1024:(q + 1) * 1024])

        def xres(g, ct):
            src = (xst[ct] if ct < 2 else xr23[ct - 2])
            return src[:, g * 512:(g + 1) * 512]

        # ---- scalar engine casts staged half-0 to fp8 ------------------
        def cast_half(eng, hh):
            for q in range(2):
                for r in range(2):
                    xs = xst[hh * 2 + r]
                    dst = xp[0][:, (r % 2) * N + hh * NH + q * 1024:
                                (r % 2) * N + hh * NH + (q + 1) * 1024]
                    src = xs[:, q * 1024:(q + 1) * 1024]
                    if eng == "act":
                        nc.scalar.activation(dst, src, AF.Copy)
                    else:
                        nc.vector.tensor_copy(dst, src)

        cast_half("act", 0)

        # ---- constants / weight prep: plain DVE casts (host transposed)
        onesDR = const.tile([128, 256], FP8E4, tag="onesDR")
        nc.gpsimd.memset(onesDR[:], 1.0)
        sel4 = const.tile([128, 128], F32, tag="sel4")
        nc.gpsimd.memset(sel4[:], 0.0)
        for c4 in range(4):
            nc.gpsimd.memset(sel4[c4 * 32:c4 * 32 + 1, :], 1.0)
        dsb = const.tile([128, 512], F32, tag="dsb")
        nc.gpsimd.memset(dsb[:], 0.0)

        wqq = const.tile([128, 512], FP8E4, tag="wqq")
        nc.vector.tensor_copy(wqq[:], wqk_f[:, 0:512])
        wkk = const.tile([128, 512], FP8E4, tag="wkk")
        nc.vector.tensor_copy(wkk[:], wqk_f[:, 512:1024])

        # ---- QK + V projection, chasing the DMAs; q/k written directly
        #      into both partition halves of their score operands ---------
        q2 = data.tile([128, NH], BF16, tag="q2")
        k2 = data.tile([128, N], BF16, tag="k2")
        vP = [data.tile([128, 2 * VPAD], FP8E4, tag=f"vP{j}", name=f"vP{j}")
              for j in range(NJ)]
        wvTp = [const.tile([128, 1024], FP8E4, tag=f"wvTp{pc}",
                           name=f"wvTp{pc}") for pc in range(2)]

        def score_pair(expP_list, g, j):
            mA, mB = 2 * j, 2 * j + 1
            q_lo = q2[0:CQ, g * 512:(g + 1) * 512]
            q_hi = q2[CQ:128, g * 512:(g + 1) * 512]
            ps = ps_sc.tile([128, 1024], F32, tag="sc", name=f"ps{g}_{j}")
            nc.tensor.matmul(
                ps[:, 0:512], k2[0:CQ, mA * 128:(mA + 1) * 128], q_lo,
                start=True, stop=True,
            )
            nc.tensor.matmul(
                ps[:, 512:1024],
                k2[CQ:128, mB * 128:(mB + 1) * 128], q_hi,
                start=True, stop=True,
            )
            nc.scalar.activation(expP_list[j][:], ps[:], AF.Exp,
                                 bias=nbias, scale=ESCALE)

        def qk_half(hh):
            for gg in range(4):
                g = hh * 4 + gg
                cols = slice(g * 512, (g + 1) * 512)
                targets = [(wkk, k2[:, cols], cst_s[:, 7:8])]
                if hh == 0:
                    targets.append((wqq, q2[:, cols], cst_s[:, 0:1]))
                for wt, dst, bias in targets:
                    ps = ps_av.tile([128, 512], F32, tag="av",
                                    name=f"qk{g}_{dst.tensor.name}")
                    for pc in range(2):
                        nc.tensor.matmul(
                            ps[:],
                            wt[:, pc * 256:(pc + 1) * 256].rearrange(
                                "p (i n) -> p i n", i=2),
                            xp[pc][:].rearrange("p (i n) -> p i n", i=2)[
                                :, :, g * 512:(g + 1) * 512],
                            start=(pc == 0), stop=(pc == 1), perf_mode=DR,
                        )
                    nc.vector.tensor_scalar_add(dst, ps[:], bias)

        def vproj_half(hh):
            for mt in range(hh * 16, hh * 16 + 16):
                ps = ps_d.tile([128, C], F32, tag="d", name=f"vps{mt}")
                for pc in range(2):
                    nc.tensor.matmul(
                        ps[:],
                        xp[pc][:].rearrange("p (i n) -> p i n", i=2)[
                            :, :, mt * 128:(mt + 1) * 128],
                        wvTp[pc][:].rearrange("p (i n) -> p i n", i=2),
                        start=(pc == 0), stop=(pc == 1), perf_mode=DR,
                    )
                j, half = divmod(mt, 2)
                nc.vector.tensor_copy(vP[j][:, half * VPAD:half * VPAD + C],
                                      ps[:])

        expP = [data.tile([128, 1024], FP8E5, tag=f"expP{j}",
                          name=f"expP{j}_0", bufs=2) for j in range(NJ)]

        qk_half(0)
        for j in range(NJ // 2):
            score_pair(expP, 0, j)

        # wvT fp8 casts (host transposed; lands mid-load on gpsimd ring)
        for pc in range(2):
            nc.vector.tensor_copy(wvTp[pc][:],
                                  wvT_f[:, pc * 1024:(pc + 1) * 1024])
        vproj_half(0)

        cast_half("dve", 1)
        qk_half(1)
        for j in range(NJ // 2, NJ):
            score_pair(expP, 0, j)
        vproj_half(1)

        # ---- attention -------------------------------------------------
        def alloc_expP(g):
            return [data.tile([128, 1024], FP8E5, tag=f"expP{j}",
                              name=f"expP{j}_{g}", bufs=2)
                    for j in range(NJ)]

        yv = y.rearrange("(ct p) n -> p ct n", ct=4)
        for g in range(N_G):
            nxt = alloc_expP(g + 1) if g + 1 < N_G else None

            # denominators: 4 concurrent M=1 col-group chains over the
            # 32 exp half-tiles, then one selector matmul sums the 4
            # partials and broadcasts d[n] to all 128 partitions.
            dsum4 = ps_d.tile([128, 512], F32, tag="d", name=f"dsum{g}")
            for s in range(8):
                for c4 in range(4):
                    j, i = divmod(c4 * 8 + s, 2)
                    nc.tensor.matmul(
                        dsum4[c4 * 32:c4 * 32 + 1, :],
                        onesDR[:, 0:1],
                        expP[j][:, i * 512:(i + 1) * 512],
                        start=(s == 0), stop=(s == 7),
                        tile_position=(0, c4 * 32),
                    )
            for c4 in range(4):
                nc.vector.tensor_copy(dsb[c4 * 32:c4 * 32 + 1, :],
                                      dsum4[c4 * 32:c4 * 32 + 1, :])
            dbc = ps_d.tile([128, 512], F32, tag="d", name=f"dbc{g}")
            nc.tensor.matmul(dbc[:], sel4[:], dsb[:], start=True, stop=True)
            dinv = data.tile([128, 512], F32, tag="dinv", bufs=2,
                             name=f"dinv{g}")
            nc.vector.reciprocal_approx_fast(dinv[:], dbc[:])
            nc.vector.tensor_scalar_mul(dinv[:], dinv[:], gsc_s)

            yow = outp.tile([128, 2048], F32, tag="yo", name=f"yo{g}")
            for ct in range(4):
                # interleave next group's scores to keep ACT fed
                if nxt is not None:
                    for j in range(ct * 4, ct * 4 + 4):
                        score_pair(nxt, g + 1, j)
                av = ps_av.tile([128, 512], F32, tag="av",
                                name=f"av{g}_{ct}")
                for j in range(NJ):
                    nc.tensor.matmul(
                        av[:],
                        vP[j][:].rearrange("p (i n) -> p i n", i=2)[
                            :, :, ct * 128:(ct + 1) * 128],
                        expP[j][:].rearrange("p (i n) -> p i n", i=2),
                        start=(j == 0), stop=(j == NJ - 1), perf_mode=DR,
                    )
                t0 = outp.tile([128, 512], F32, tag="ysc")
                nc.vector.tensor_mul(t0[:], av[:], dinv[:])
                yslice = yow[:, ct * 512:(ct + 1) * 512]
                nc.vector.scalar_tensor_tensor(
                    yslice, t0[:], cst_s[:, 2 + ct:3 + ct], xres(g, ct),
                    ALU.add, ALU.add)
                if g == N_G - 1:
                    # last group: drain per tile
                    nc.sync.dma_start(
                        y[ct * 128:(ct + 1) * 128, g * 512:(g + 1) * 512],
                        yslice)
            if g < N_G - 1:
                nc.sync.dma_start(
                    yv[:, :, g * 512:(g + 1) * 512],
                    yow[:].rearrange("p (ct n) -> p ct n", ct=4))
            expP = nxt


def build_nc():
    nc = bacc.Bacc("TRN2", target_bir_lowering=False, debug=False,
                   num_devices=NCORES)
    x = nc.dram_tensor("x", [C, N], F32, kind="ExternalInput")
    wqkT = nc.dram_tensor("wqkT", [128, 1024], BF16, kind="ExternalInput")
    wvT = nc.dram_tensor("wvT", [128, 2048], BF16, kind="ExternalInput")
    cst = nc.dram_tensor("cst", [128, 8], F32, kind="ExternalInput")
    y = nc.dram_tensor("y", [C, NH], F32, kind="ExternalOutput")
    with tile.TileContext(nc) as tc:
        _emit(tc, x.ap(), wqkT.ap(), wvT.ap(), cst.ap(), y.ap())
    nc.compile()
    return nc


def make_in_maps(inputs):
    xf = np.ascontiguousarray(
        np.asarray(inputs["x"], dtype=np.float32).reshape(B, C, N))
    # WSCALE pre-scaling keeps the fp8e4 weights out of the subnormal
    # range; the score-side 256x is undone by the exp activation scale,
    # the V-side 16x by dividing gamma. Weights are host-transposed into
    # the DoubleRow stationary layout [p, pc*F + i*(F//2) + m] =
    # w[m, pc*256 + i*128 + p], with q (and k) duplicated across both
    # output column halves so each lands in both score-operand halves.
    wq = np.asarray(inputs["wq"], dtype=np.float32) * WSCALE
    wk = np.asarray(inputs["wk"], dtype=np.float32) * WSCALE
    wv = np.asarray(inputs["wv"], dtype=np.float32) * WSCALE

    def dr_layout(w):
        # w: [M, 512] -> [128, 2, 2, M]: [p, pc, i, m] = w[m, pc*256+i*128+p]
        return np.ascontiguousarray(
            w.T.reshape(2, 2, 128, w.shape[0]).transpose(2, 0, 1, 3)
            .reshape(128, -1))

    import ml_dtypes
    wqkT = np.concatenate(
        [dr_layout(np.concatenate([wq, wq], axis=0)),
         dr_layout(np.concatenate([wk, wk], axis=0))], axis=1
    ).astype(ml_dtypes.bfloat16)
    wvT = dr_layout(wv).astype(ml_dtypes.bfloat16)
    gamma_v = float(np.asarray(inputs["gamma"], dtype=np.float32).reshape(()))
    cst = np.zeros((128, 8), dtype=np.float32)
    bq2 = np.concatenate([np.asarray(inputs["bq"], dtype=np.float32)] * 2)
    bk2 = np.concatenate([np.asarray(inputs["bk"], dtype=np.float32)] * 2)
    cst[:, 0] = bq2 * WSCALE
    cst[:, 7] = bk2 * WSCALE
    cst[:, 1] = gamma_v / WSCALE
    # v-bias contributes exactly gamma*bv to y (softmax rows sum to 1)
    cst[:, 2:6] = (gamma_v * np.asarray(inputs["bv"], dtype=np.float32)
                   ).reshape(4, 128).T
    cst[:, 6] = -4.0
    in_maps = []
    for i in range(NCORES):
        b, h = divmod(i, 2)
        xr = np.roll(xf[b], -h * NH, axis=1) if h else xf[b]
        in_maps.append({
            "x": np.ascontiguousarray(xr), "wqkT": wqkT, "wvT": wvT,
            "cst": cst,
        })
    return in_maps


_NC = None


def _get_nc():
    global _NC
    if _NC is None:
        _NC = build_nc()
    return _NC


def kernel(**inputs):
    nc = _get_nc()
    in_maps = make_in_maps(inputs)
    res = bass_utils.run_bass_kernel_spmd(nc, in_maps, core_ids=list(range(NCORES)))
    yf = np.empty((B, C, N), dtype=np.float32)
    for i in range(NCORES):
        b, h = divmod(i, 2)
        yf[b][:, h * NH:(h + 1) * NH] = res.results[i]["y"]
    return yf.reshape(B, C, W, H)
